# revision 6
# baseline (speedup 1.0000x reference)
"""CRNN (conv3x3 -> ReLU -> freq-maxpool -> GRU scan -> FC) on 8 Trainium2
NeuronCores, data-parallel over batch.

Per-core layout (NB = 8 batch items per core):
  - conv: banded-weight matmuls over the frequency contraction (K=64/128),
    time shifts via column offsets into a padded [128, T+2] tile; two
    accumulating matmuls per f-pair produce PSUM [128=2f x 64c, 512t];
    running tensor_max over f-pairs + ReLU(+bias) gives feat[c, t].
  - feat is written batch-interleaved into bigU[64:128, t*NB + b].
  - xn = W_ihn @ feat + b_ihn precomputed for all t (PE).
  - GRU scan: per step k, two matmuls (rz gates with [h; feat] K=128 rhs,
    hn with K=64), sigmoid w/ bias, scalar_tensor_tensor, tanh, and the
    h-update on DVE. h_k kept in bigU[0:64, k*NB:(k+1)*NB].
  - FC: fc_w @ hs for all t (PE), then strided DMA out.
"""

import os
import numpy as np

import concourse.bass as bass
import concourse.mybir as mybir
import concourse.tile as tile
from concourse import bacc
from concourse.bass_utils import run_bass_kernel_spmd

F32 = mybir.dt.float32
AF = mybir.ActivationFunctionType
OP = mybir.AluOpType

B, F, T = 64, 64, 1024
C = 64          # conv channels == rnn input size
H = 64          # rnn hidden
OUT = 2
NCORES = 8
NB = B // NCORES  # batch per core
NFP = F // 2      # f-pairs


def build_crnn(nb=NB, t_steps=T):
    """Build the SPMD per-core bass module."""
    nc = bacc.Bacc("TRN2", target_bir_lowering=False, debug=False)
    TB = t_steps * nb
    NTH = t_steps // 512 if t_steps >= 512 else 1
    THW = min(512, t_steps)   # t-half width
    NJ = TB // 512 if TB >= 512 else 1
    JW = min(512, TB)

    x_d = nc.declare_dram_parameter("x", [nb, F, t_steps], F32, isOutput=False)
    convA_d = nc.declare_dram_parameter("convA", [128, NFP * 128], F32, isOutput=False)
    convB_d = nc.declare_dram_parameter("convB", [64, NFP * 128], F32, isOutput=False)
    cb_d = nc.declare_dram_parameter("conv_bias", [C, 1], F32, isOutput=False)
    wrz_d = nc.declare_dram_parameter("w_rz_lhsT", [128, 128], F32, isOutput=False)
    wn_d = nc.declare_dram_parameter("w_n_lhsT", [H, H], F32, isOutput=False)
    win_d = nc.declare_dram_parameter("w_in_lhsT", [C, H], F32, isOutput=False)
    brz_d = nc.declare_dram_parameter("b_rz", [128, 1], F32, isOutput=False)
    bhn_d = nc.declare_dram_parameter("b_hn", [H, 1], F32, isOutput=False)
    bin_d = nc.declare_dram_parameter("b_in_row", [1, H], F32, isOutput=False)
    fcw_d = nc.declare_dram_parameter("fc_lhsT", [H, OUT], F32, isOutput=False)
    fcb_d = nc.declare_dram_parameter("fc_b_row", [1, OUT], F32, isOutput=False)
    out_d = nc.declare_dram_parameter("out", [nb, OUT, t_steps], F32, isOutput=True)

    with tile.TileContext(nc) as tc:
        with (
            tc.tile_pool(name="persist", bufs=1) as persist,
            tc.tile_pool(name="x2pool", bufs=2) as x2p,
            tc.tile_pool(name="work", bufs=4) as work,
            tc.tile_pool(name="scanw", bufs=3) as scanw,
            tc.tile_pool(name="pp_conv", bufs=2, space="PSUM") as ppc,
            tc.tile_pool(name="pp_scan", bufs=2, space="PSUM") as pps,
            tc.tile_pool(name="pp_misc", bufs=2, space="PSUM") as ppm,
        ):
            convA = persist.tile([128, NFP * 128], F32)
            convB = persist.tile([64, NFP * 128], F32)
            cb = persist.tile([C, 1], F32)
            w_rz = persist.tile([128, 128], F32)
            w_n = persist.tile([H, H], F32)
            w_in_full = persist.tile([128, H], F32)  # W_in content in rows 64:128
            w_in = w_in_full[64:128, :]
            b_rz = persist.tile([128, 1], F32)
            b_hn = persist.tile([H, 1], F32)
            b_in = persist.tile([1, H], F32)
            fc_w = persist.tile([H, OUT], F32)
            fc_b = persist.tile([1, OUT], F32)
            ones = persist.tile([1, JW], F32)
            bigU = persist.tile([128, (t_steps + 1) * nb], F32)
            xn_sb = persist.tile([H, TB], F32)
            out_sb = persist.tile([OUT, TB], F32)

            nc.sync.dma_start(out=convA, in_=convA_d[:, :])
            nc.sync.dma_start(out=convB, in_=convB_d[:, :])
            nc.sync.dma_start(out=cb, in_=cb_d[:, :])
            nc.sync.dma_start(out=w_rz, in_=wrz_d[:, :])
            nc.sync.dma_start(out=w_n, in_=wn_d[:, :])
            nc.sync.dma_start(out=w_in, in_=win_d[:, :])
            nc.sync.dma_start(out=b_rz, in_=brz_d[:, :])
            nc.sync.dma_start(out=b_hn, in_=bhn_d[:, :])
            nc.sync.dma_start(out=b_in, in_=bin_d[:, :])
            nc.sync.dma_start(out=fc_w, in_=fcw_d[:, :])
            nc.sync.dma_start(out=fc_b, in_=fcb_d[:, :])
            nc.vector.memset(ones, 1.0)
            nc.vector.memset(bigU[0:64, 0:nb], 0.0)   # h_0 = 0

            # ---------------- conv + freq max ----------------
            for b in range(nb):
                X2 = x2p.tile([128, t_steps + 2], F32, tag="x2")
                nc.sync.dma_start(out=X2[0:64, 1 : t_steps + 1], in_=x_d[b, :, :])
                nc.sync.dma_start(out=X2[64:128, 0:t_steps], in_=x_d[b, :, :])
                nc.vector.memset(X2[0:64, 0:1], 0.0)
                nc.vector.memset(X2[0:64, t_steps + 1 : t_steps + 2], 0.0)
                nc.vector.memset(X2[64:128, t_steps : t_steps + 2], 0.0)

                for th in range(NTH):
                    macc = work.tile([128, THW], F32, tag="macc", name="macc")
                    for fp in range(NFP):
                        ps = ppc.tile([128, THW], F32, tag="cps", name="cps")
                        nc.tensor.matmul(
                            ps, convA[:, fp * 128 : (fp + 1) * 128],
                            X2[:, th * THW : th * THW + THW],
                            start=True, stop=False,
                        )
                        nc.tensor.matmul(
                            ps, convB[:, fp * 128 : (fp + 1) * 128],
                            X2[0:64, th * THW + 2 : th * THW + THW + 2],
                            start=False, stop=True,
                        )
                        if fp == 0:
                            nc.vector.tensor_copy(macc, ps)
                        else:
                            nc.vector.tensor_max(macc, macc, ps)
                    mhi = work.tile([64, THW], F32, tag="mhi", name="mhi")
                    nc.vector.tensor_copy(mhi, macc[64:128, :])
                    m2 = work.tile([64, THW], F32, tag="m2", name="m2")
                    nc.vector.tensor_max(m2, macc[0:64, :], mhi)
                    # feat[c, t] -> bigU[64+c, t*nb + b], ReLU(max + bias)
                    out_ap = bigU[64:128, th * THW * nb + b : (th * THW + THW) * nb : nb]
                    nc.scalar.activation(out_ap, m2, AF.Relu, bias=cb)

            # ---------------- xn precompute ----------------
            for j in range(NJ):
                ps = ppm.tile([H, JW], F32, tag="mps", name="xnps")
                nc.tensor.matmul(
                    ps, w_in, bigU[64:128, j * JW : (j + 1) * JW],
                    start=True, stop=False,
                )
                nc.tensor.matmul(ps, b_in, ones, start=False, stop=True)
                nc.scalar.copy(xn_sb[:, j * JW : (j + 1) * JW], ps)

            # ---------------- GRU scan ----------------
            for k in range(t_steps):
                col = slice(k * nb, (k + 1) * nb)
                ncol = slice((k + 1) * nb, (k + 2) * nb)
                psum_rz = pps.tile([128, nb], F32, tag="rz", name="rz")
                psum_hn = pps.tile([H, nb], F32, tag="hn", name="hn")
                nc.tensor.matmul(psum_rz, w_rz, bigU[:, col], start=True, stop=True)
                nc.tensor.matmul(psum_hn, w_n, bigU[0:64, col], start=True, stop=True)

                r_s = scanw.tile([H, nb], F32, tag="rs", name="rs")
                nc.scalar.activation(r_s, psum_rz[0:64, :], AF.Sigmoid, bias=b_rz[0:64, :])
                z_s = scanw.tile([H, nb], F32, tag="zs", name="zs")
                nc.scalar.activation(z_s, psum_rz[64:128, :], AF.Sigmoid, bias=b_rz[64:128, :])
                q = scanw.tile([H, nb], F32, tag="q", name="q")
                nc.vector.scalar_tensor_tensor(
                    out=q, in0=psum_hn, scalar=b_hn, in1=r_s,
                    op0=OP.add, op1=OP.mult,
                )
                q2 = scanw.tile([H, nb], F32, tag="q2", name="q2")
                nc.vector.tensor_add(q2, q, xn_sb[:, col])
                n_t = scanw.tile([H, nb], F32, tag="n", name="n")
                nc.scalar.activation(n_t, q2, AF.Tanh)
                d_t = scanw.tile([H, nb], F32, tag="d", name="d")
                nc.vector.tensor_sub(d_t, bigU[0:64, col], n_t)
                e_t = scanw.tile([H, nb], F32, tag="e", name="e")
                nc.vector.tensor_mul(e_t, d_t, z_s)
                nc.vector.tensor_add(bigU[0:64, ncol], e_t, n_t)

            # ---------------- FC ----------------
            for j in range(NJ):
                ps = ppm.tile([OUT, JW], F32, tag="mps", name="fcps")
                nc.tensor.matmul(
                    ps, fc_w, bigU[0:64, nb + j * JW : nb + (j + 1) * JW],
                    start=True, stop=False,
                )
                nc.tensor.matmul(ps, fc_b, ones, start=False, stop=True)
                nc.scalar.copy(out_sb[:, j * JW : (j + 1) * JW], ps)

            # ---------------- output DMA ----------------
            for b in range(nb):
                nc.sync.dma_start(
                    out=out_d[b, :, :],
                    in_=out_sb[:, b : TB : nb],
                )

    nc.finalize()
    return nc


def prep_weights(conv_w, conv_b, w_ih, w_hh, b_ih, b_hh, fc_w, fc_b):
    """Host-side rearrangement of the small weights into device layouts."""
    conv_w = np.asarray(conv_w, np.float32)
    A = np.zeros((128, NFP * 128), np.float32)
    Bm = np.zeros((64, NFP * 128), np.float32)
    for fp in range(NFP):
        for fo in range(2):
            fout = 2 * fp + fo
            for fprime in range(max(0, fout - 1), min(64, fout + 2)):
                i = fprime - fout + 1
                cols = slice(fp * 128 + fo * 64, fp * 128 + fo * 64 + 64)
                A[fprime, cols] = conv_w[:, 0, i, 0]
                A[64 + fprime, cols] = conv_w[:, 0, i, 1]
                Bm[fprime, cols] = conv_w[:, 0, i, 2]
    w_ih = np.asarray(w_ih, np.float32)
    w_hh = np.asarray(w_hh, np.float32)
    b_ih = np.asarray(b_ih, np.float32)
    b_hh = np.asarray(b_hh, np.float32)
    return {
        "convA": A,
        "convB": Bm,
        "conv_bias": np.asarray(conv_b, np.float32).reshape(C, 1),
        "w_rz_lhsT": np.concatenate(
            [w_hh[0:128, :].T, w_ih[0:128, :].T], axis=0
        ).astype(np.float32).copy(),
        "w_n_lhsT": w_hh[128:192, :].T.astype(np.float32).copy(),
        "w_in_lhsT": w_ih[128:192, :].T.astype(np.float32).copy(),
        "b_rz": (b_ih[0:128] + b_hh[0:128]).reshape(128, 1).astype(np.float32),
        "b_hn": b_hh[128:192].reshape(H, 1).astype(np.float32),
        "b_in_row": b_ih[128:192].reshape(1, H).astype(np.float32),
        "fc_lhsT": np.asarray(fc_w, np.float32).T.copy(),
        "fc_b_row": np.asarray(fc_b, np.float32).reshape(1, OUT),
    }


_NC_CACHE = {}


def _get_nc():
    if "nc" not in _NC_CACHE:
        _NC_CACHE["nc"] = build_crnn()
    return _NC_CACHE["nc"]


def run(inputs, trace=False):
    """Returns (out [B, OUT, T], BassKernelResults)."""
    x = np.asarray(inputs["x"], np.float32)
    wd = prep_weights(
        inputs["conv_w"], inputs["conv_b"], inputs["w_ih"], inputs["w_hh"],
        inputs["b_ih"], inputs["b_hh"], inputs["fc_w"], inputs["fc_b"],
    )
    nc = _get_nc()
    in_maps = []
    for i in range(NCORES):
        m = dict(wd)
        m["x"] = np.ascontiguousarray(x[i * NB : (i + 1) * NB])
        in_maps.append(m)
    res = run_bass_kernel_spmd(nc, in_maps, list(range(NCORES)), trace=trace)
    out = np.concatenate([res.results[i]["out"] for i in range(NCORES)], axis=0)
    return out, res


def kernel(**inputs) -> np.ndarray:
    out, _ = run(inputs, trace=False)
    return out
